# revision 1
# baseline (speedup 1.0000x reference)
"""AerialPatchSampler Trainium2 kernel (v2).

Samples N rotated/scaled/translated 64x64 patches from a (C=64, 512, 512)
aerial feature image with bilinear interpolation (grid_sample semantics,
align_corners=False, zeros padding + validity mask).

Sharding: 8 cores; core k handles batch b = k//4 and hypotheses
n in [32*(k%4), 32*(k%4)+32).  Each core receives its batch's full image.

Per-core plan:
  Phase A: build qimg[(Hp*Wp), 2*C] in DRAM in bf16: qimg[(y*Wp+x)] holds
           the channel vectors of padded rows y-1 and y at padded column x
           (zero border = grid_sample zero padding).  Built from 32-row
           tiles: PE-transpose (row,chan)->(col,chan) blocks, pack rows
           row-major in SBUF, then store 31 overlapping double-row slots
           per tile with one DMA per column block.
  Phase B: coordinates/weights/indices for ALL patches computed in a few
           batched [128, 1024] ops (pixel p = g*128 + part, g = patch-local
           block).  Per patch: ONE batched indirect DMA (4096 i32 indices,
           each reading 512B = all 4 bilinear corners in bf16); bf16
           weighted combine with channel-broadcast (stride-0) APs writing
           interleaved columns 128*t + 2*c + H; PE-transpose back to
           channel-major (partition = 2c+H) so the final store per patch
           is 128 descriptors of 8KB contiguous runs.
"""

import sys

for _p in ("/opt/trn_rl_repo", "/root/.axon_site/_ro/trn_rl_repo"):
    if _p not in sys.path:
        sys.path.insert(0, _p)

import numpy as np

import concourse.bass as bass
import concourse.tile as tile
from concourse import bacc, mybir
from concourse.bass import AP
from concourse.masks import make_identity

F32 = mybir.dt.float32
BF16 = mybir.dt.bfloat16
I32 = mybir.dt.int32
ALU = mybir.AluOpType
ACTF = mybir.ActivationFunctionType

B, C, H, W = 2, 64, 512, 512
N = 128
HB, WB = 64, 64
NCORES = 8
NP = N // (NCORES // B)  # 32 patches per core
Hp, Wp = H + 2, W + 2  # 514, zero-padded
PV = Hp * Wp
PIX = HB * WB  # 4096
QR = 2 * C  # 128 elements per q slot
MAGIC = 12582912.0  # 1.5 * 2^23: RNE-to-int trick, ULP 1.0 for |t| < 2^22
RT = 32  # image rows per Phase-A tile
NT = H // RT  # 16 tiles


def _ap(base: AP, extra_off: int, dims) -> AP:
    return AP(base.tensor, base.offset + extra_off, [list(d) for d in dims])


def build_program(repeat=1):
    nc = bacc.Bacc(
        "TRN2",
        target_bir_lowering=False,
        debug=False,
        enable_asserts=False,
    )
    img = nc.dram_tensor("img", [C, H, W], F32, kind="ExternalInput").ap()
    pose = nc.dram_tensor("pose", [NP, 3], F32, kind="ExternalInput").ap()
    osc = nc.dram_tensor("osc", [1, 1], F32, kind="ExternalInput").ap()
    out = nc.dram_tensor("out", [NP, C, PIX], F32, kind="ExternalOutput").ap()
    qimg = nc.dram_tensor("qimg", [PV, QR], BF16, kind="Internal").ap()

    with tile.TileContext(nc, trace_sim=False) as tc:
      for _rep in range(repeat):
        with tc.tile_pool(name="const", bufs=1) as cpool:
            ident = cpool.tile([128, 128], F32)
            make_identity(nc, ident[:])
            identb = cpool.tile([128, 128], BF16)
            nc.vector.tensor_copy(identb[:], ident[:])

            zt = cpool.tile([128, 257], BF16)
            nc.vector.memset(zt[:], 0.0)

            # ---- params broadcast to all partitions ----
            u_bc = cpool.tile([128, NP], F32)
            v_bc = cpool.tile([128, NP], F32)
            th_bc = cpool.tile([128, NP], F32)
            nc.sync.dma_start(u_bc[:], _ap(pose, 0, [[0, 128], [3, NP]]))
            nc.sync.dma_start(v_bc[:], _ap(pose, 1, [[0, 128], [3, NP]]))
            nc.sync.dma_start(th_bc[:], _ap(pose, 2, [[0, 128], [3, NP]]))
            os_bc = cpool.tile([128, 1], F32)
            nc.sync.dma_start(os_bc[:], _ap(osc, 0, [[0, 128], [1, 1]]))

            # cos(-th) = sin(pi/2 - |th|);  sin(-th) = sin(th * -1)
            zbias = cpool.tile([128, 1], F32)
            nc.vector.memset(zbias[:], 0.0)
            pibias = cpool.tile([128, 1], F32)
            nc.vector.memset(pibias[:], 1.5707963267948966)
            ebias = cpool.tile([128, 1], F32)
            nc.vector.memset(ebias[:], -255.5)
            sin_bc = cpool.tile([128, NP], F32)
            cos_bc = cpool.tile([128, NP], F32)
            abs_th = cpool.tile([128, NP], F32)
            nc.scalar.activation(
                sin_bc[:], th_bc[:], ACTF.Sin, scale=-1.0, bias=zbias[:, 0:1]
            )
            nc.scalar.activation(abs_th[:], th_bc[:], ACTF.Abs, bias=zbias[:, 0:1])
            nc.scalar.activation(
                cos_bc[:], abs_th[:], ACTF.Sin, scale=-1.0, bias=pibias[:, 0:1]
            )

            # ---- base grids ----
            # pixel (within patch) = g*128 + part; i = 2*g + part//64; j = part%64
            pi32 = cpool.tile([128, 1], I32)
            nc.gpsimd.iota(pi32[:], pattern=[[0, 1]], base=0, channel_multiplier=1)
            j32 = cpool.tile([128, 1], I32)
            nc.vector.tensor_scalar(j32[:], pi32[:], 63, None, ALU.bitwise_and)
            pd64 = cpool.tile([128, 1], I32)
            nc.vector.tensor_scalar(pd64[:], pi32[:], 6, None, ALU.arith_shift_right)
            if2 = cpool.tile([128, 32], I32)
            nc.gpsimd.iota(if2[:], pattern=[[2, 32]], base=0, channel_multiplier=0)

            jf = cpool.tile([128, 1], F32)
            nc.vector.tensor_copy(jf[:], j32[:])
            pdf = cpool.tile([128, 1], F32)
            nc.vector.tensor_copy(pdf[:], pd64[:])
            i_f = cpool.tile([128, 32], F32)
            nc.vector.tensor_copy(i_f[:], if2[:])
            nc.vector.tensor_scalar(i_f[:], i_f[:], pdf[:, 0:1], None, ALU.add)

            # gu0 = (63 - i) * os ;  gv0 = (j - 32) * os
            gu0 = cpool.tile([128, 32], F32)
            nc.vector.tensor_scalar(gu0[:], i_f[:], -1.0, 63.0, ALU.mult, ALU.add)
            nc.vector.tensor_scalar(gu0[:], gu0[:], os_bc[:, 0:1], None, ALU.mult)
            gv0 = cpool.tile([128, 1], F32)
            nc.vector.tensor_scalar(
                gv0[:], jf[:], 32.0, os_bc[:, 0:1], ALU.subtract, ALU.mult
            )

            # ---- persistent Phase-B tensors (batched over all patches) ----
            # free dim = (n, g): patch-major, 32*32 = 1024
            NG = NP * 32
            w_tl = cpool.tile([128, NG], BF16)
            w_tr = cpool.tile([128, NG], BF16)
            w_bl = cpool.tile([128, NG], BF16)
            w_br = cpool.tile([128, NG], BF16)
            it32 = cpool.tile([128, NG], I32)

            # ============ Phase A: build double-row channel-last image =====
            ztap = zt[:]

            def zfill(eng, dst_off, nblk, blk_stride, blk_len):
                done = 0
                while done < nblk:
                    cnt = min(128, nblk - done)
                    eng.dma_start(
                        _ap(
                            qimg,
                            dst_off + done * blk_stride,
                            [[blk_stride, cnt], [1, blk_len]],
                        ),
                        _ap(ztap, 0, [[257, cnt], [1, blk_len]]),
                    )
                    done += cnt

            # col pads for q rows 0..512: slot (y, 513) + slot (y+1, 0)
            zfill(nc.scalar, (Wp - 1) * QR, H + 1, Wp * QR, 2 * QR)
            zfill(nc.scalar, 0, 1, QR, QR)  # slot (0, 0)
            # q row 0 low halves (pad image row): cols 1..512
            zfill(nc.sync, QR, W, QR, C)
            # q row 512 high halves (pad image row): cols 1..512
            zfill(nc.sync, (H * Wp + 1) * QR + C, W, QR, C)
            # q row 513: never gathered, but keep DRAM finite
            zfill(nc.sync, (H + 1) * Wp * QR, Wp, QR, QR)

            with (
                tc.tile_pool(name="lpool", bufs=3) as lpool,
                tc.tile_pool(name="papsum", bufs=2, space="PSUM") as papsum,
                tc.tile_pool(name="spool", bufs=3) as spool,
            ):
                KP = RT // 2  # 16 row-pairs per tile
                prev_sb = None
                for t_i in range(NT):
                    base = t_i * RT  # rows [base, base+RT)
                    lt = lpool.tile([128, KP * 512], F32, tag="lt")
                    for y2 in (0, 1):
                        eng = nc.sync if y2 == 0 else nc.gpsimd
                        eng.dma_start(
                            lt[64 * y2 : 64 * y2 + 64, :],
                            _ap(
                                img,
                                (base + y2) * W,
                                [[H * W, C], [2 * W, KP], [1, W]],
                            ),
                        )
                    # sb[xl, k*2048 + r*64 + c] = img[c, base+r, 128k+xl], bf16
                    sb = spool.tile([128, 4 * RT * C], BF16, tag="sb")
                    for q4 in range(KP // 4):
                        bt = papsum.tile([128, 2048], F32, tag="bt")
                        for pp in range(4):
                            p = 4 * q4 + pp
                            for k in range(4):
                                nc.tensor.transpose(
                                    out=bt[:, pp * 512 + 128 * k : pp * 512 + 128 * (k + 1)],
                                    in_=lt[:, p * 512 + 128 * k : p * 512 + 128 * (k + 1)],
                                    identity=ident[:],
                                )
                        # bt[xl, pp*512 + 128k + y2*64 + c]
                        #   -> sb col 2048k + 128*(4q4+pp) + 64y2 + c
                        sap = sb[:]
                        nc.vector.tensor_copy(
                            _ap(
                                sap,
                                512 * q4,
                                [sap.ap[0], [2048, 4], [128, 4], [1, 128]],
                            ),
                            _ap(bt[:], 0, [bt[:].ap[0], [128, 4], [512, 4], [1, 128]]),
                        )
                    sap = sb[:]
                    # whole slots y = base+1 .. base+31 (covers rows y-1, y)
                    store_engs = (nc.sync, nc.scalar, nc.scalar, nc.gpsimd)
                    for k in range(4):
                        eng = store_engs[k]
                        eng.dma_start(
                            _ap(
                                qimg,
                                ((base + 1) * Wp + 1 + 128 * k) * QR,
                                [[QR, 128], [Wp * QR, RT - 1], [1, QR]],
                            ),
                            _ap(
                                sap,
                                k * 2048,
                                [sap.ap[0], [64, RT - 1], [1, QR]],
                            ),
                        )
                    # boundary slot y = base: low half from prev tile row 31,
                    # high half from this tile row 0
                    if prev_sb is not None:
                        pap = prev_sb[:]
                        nc.sync.dma_start(
                            _ap(
                                qimg,
                                (base * Wp + 1) * QR,
                                [[QR, 128], [128 * QR, 4], [1, C]],
                            ),
                            _ap(pap, (RT - 1) * 64, [pap.ap[0], [2048, 4], [1, C]]),
                        )
                    nc.sync.dma_start(
                        _ap(
                            qimg,
                            (base * Wp + 1) * QR + C,
                            [[QR, 128], [128 * QR, 4], [1, C]],
                        ),
                        _ap(sap, 0, [sap.ap[0], [2048, 4], [1, C]]),
                    )
                    if t_i == NT - 1:
                        # slot 512 low half = img row 511 = last row
                        nc.sync.dma_start(
                            _ap(
                                qimg,
                                (H * Wp + 1) * QR,
                                [[QR, 128], [128 * QR, 4], [1, C]],
                            ),
                            _ap(sap, (RT - 1) * 64, [sap.ap[0], [2048, 4], [1, C]]),
                        )
                    prev_sb = sb

            # ========== Phase B coords: batched over all patches ===========
            # free dim layout: (n, g) patch-major; views below
            with tc.tile_pool(name="coord", bufs=1) as crd:
                def nv(t):  # broadcast per-patch scalar over g
                    a = t[:]
                    return _ap(a, 0, [a.ap[0], [1, NP], [0, 32]])

                def gv(t):  # broadcast per-(p,g) grid over n
                    a = t[:]
                    return _ap(a, 0, [a.ap[0], [0, NP], [1, 32]])

                def gv1(t):  # broadcast per-p grid over (n, g)
                    a = t[:]
                    return _ap(a, 0, [a.ap[0], [0, NP], [0, 32]])

                # grid_u = (u + cos*gu0) - sin*gv0 ; grid_v = (v+sin*gu0)+cos*gv0
                xu = crd.tile([128, NG], F32)
                nc.vector.tensor_tensor(xu[:], gv(gu0), nv(cos_bc), ALU.mult)
                nc.vector.tensor_tensor(xu[:], xu[:], nv(u_bc), ALU.add)
                t3 = crd.tile([128, NG], F32)
                nc.vector.tensor_tensor(t3[:], gv1(gv0), nv(sin_bc), ALU.mult)
                nc.vector.tensor_tensor(xu[:], xu[:], t3[:], ALU.subtract)

                yv = crd.tile([128, NG], F32)
                nc.vector.tensor_tensor(yv[:], gv(gu0), nv(sin_bc), ALU.mult)
                nc.vector.tensor_tensor(yv[:], yv[:], nv(v_bc), ALU.add)
                nc.vector.tensor_tensor(t3[:], gv1(gv0), nv(cos_bc), ALU.mult)
                nc.vector.tensor_tensor(yv[:], yv[:], t3[:], ALU.add)

                # valid: -0.5 < coord < 511.5  <=>  |coord - 255.5| < 256
                ax = crd.tile([128, NG], F32)
                nc.scalar.activation(ax[:], xu[:], ACTF.Abs, bias=ebias[:, 0:1])
                vx = crd.tile([128, NG], F32)
                nc.vector.tensor_scalar(vx[:], ax[:], 256.0, None, ALU.is_lt)
                nc.scalar.activation(ax[:], yv[:], ACTF.Abs, bias=ebias[:, 0:1])
                valid = crd.tile([128, NG], F32)
                nc.vector.tensor_scalar(valid[:], ax[:], 256.0, None, ALU.is_lt)
                nc.vector.tensor_tensor(valid[:], valid[:], vx[:], ALU.mult)

                # floor via RNE(x - 0.5) with the 1.5*2^23 trick
                def floor_frac(src_t, dst_f, dst_w):
                    m = crd.tile([128, NG], F32, tag="ff_m")
                    nc.vector.tensor_scalar(
                        m[:], src_t[:], -0.5, MAGIC, ALU.add, ALU.add
                    )
                    nc.vector.tensor_scalar(dst_f[:], m[:], -MAGIC, None, ALU.add)
                    nc.vector.tensor_tensor(
                        dst_w[:], src_t[:], dst_f[:], ALU.subtract
                    )

                x0f = crd.tile([128, NG], F32)
                wx = crd.tile([128, NG], F32)
                y0f = crd.tile([128, NG], F32)
                wy = crd.tile([128, NG], F32)
                floor_frac(xu, x0f, wx)
                floor_frac(yv, y0f, wy)

                # weights (valid folded into the y-terms)
                wyb = crd.tile([128, NG], F32)
                nc.vector.tensor_scalar(wyb[:], wy[:], -1.0, 1.0, ALU.mult, ALU.add)
                wA = crd.tile([128, NG], F32)
                nc.vector.tensor_tensor(wA[:], wyb[:], valid[:], ALU.mult)
                wB = crd.tile([128, NG], F32)
                nc.vector.tensor_tensor(wB[:], wy[:], valid[:], ALU.mult)
                wxb = crd.tile([128, NG], F32)
                nc.vector.tensor_scalar(wxb[:], wx[:], -1.0, 1.0, ALU.mult, ALU.add)
                tmpw = crd.tile([128, NG], F32)
                nc.vector.tensor_tensor(tmpw[:], wxb[:], wA[:], ALU.mult)
                nc.vector.tensor_copy(w_tl[:], tmpw[:])
                nc.vector.tensor_tensor(tmpw[:], wx[:], wA[:], ALU.mult)
                nc.vector.tensor_copy(w_tr[:], tmpw[:])
                nc.vector.tensor_tensor(tmpw[:], wxb[:], wB[:], ALU.mult)
                nc.vector.tensor_copy(w_bl[:], tmpw[:])
                nc.vector.tensor_tensor(tmpw[:], wx[:], wB[:], ALU.mult)
                nc.vector.tensor_copy(w_br[:], tmpw[:])

                # gather slot index: ((y0+1)*Wp + (x0+1)) * valid
                sx = crd.tile([128, NG], F32)
                nc.vector.tensor_scalar(sx[:], x0f[:], 1.0, None, ALU.add)
                ry = crd.tile([128, NG], F32)
                nc.vector.tensor_scalar(ry[:], y0f[:], 1.0, None, ALU.add)
                idxr = crd.tile([128, NG], F32)
                nc.vector.scalar_tensor_tensor(
                    idxr[:], ry[:], float(Wp), sx[:], ALU.mult, ALU.add
                )
                nc.vector.tensor_tensor(idxr[:], idxr[:], valid[:], ALU.mult)
                nc.vector.tensor_copy(it32[:], idxr[:])

            # ================= Phase B: per-patch sample ===================
            with (
                tc.tile_pool(name="gpool", bufs=4) as gpool,
                tc.tile_pool(name="fpool", bufs=3) as fpool,
                tc.tile_pool(name="tpsum", bufs=3, space="PSUM") as tpsum,
                tc.tile_pool(name="opool", bufs=3) as opool,
            ):
                for n in range(NP):
                    # gathers: HW indirect DMA semantics allow ONE index per
                    # destination partition, reading the dest row length
                    # (256 bf16 = 512B = all 4 corners of pixel g*128 + p)
                    ga = gpool.tile([128, 32 * 256], BF16, tag="ga")
                    gap = ga[:]
                    for g in range(32):
                        nc.gpsimd.indirect_dma_start(
                            out=_ap(gap, g * 256, [gap.ap[0], [1, 256]]),
                            out_offset=None,
                            in_=qimg,
                            in_offset=bass.IndirectOffsetOnAxis(
                                ap=it32[:, 32 * n + g : 32 * n + g + 1], axis=0
                            ),
                        )

                    # combine: ft[p, 128t + 2c + H] = sum_k corner_k * w_k
                    # gathered elem e = x*128 + h*64 + c (x: col corner,
                    # h: row corner); g = 16H + t
                    ft = fpool.tile([128, 2048], BF16, tag="ft")
                    ftap = ft[:]
                    f3 = _ap(
                        ftap, 0, [ftap.ap[0], [1, 2], [128, 16], [2, 64]]
                    )

                    def corner(off):
                        return _ap(
                            gap, off, [gap.ap[0], [4096, 2], [256, 16], [1, 64]]
                        )

                    def wb_(w_t):
                        wap = w_t[:]
                        return _ap(
                            wap, 32 * n, [wap.ap[0], [16, 2], [1, 16], [0, 64]]
                        )

                    tmp = fpool.tile([128, 2048], BF16, tag="tmp")
                    tmap = tmp[:]
                    t3v = _ap(
                        tmap, 0, [tmap.ap[0], [1, 2], [128, 16], [2, 64]]
                    )
                    nc.vector.tensor_tensor(f3, corner(0), wb_(w_tl), ALU.mult)
                    nc.vector.tensor_tensor(t3v, corner(128), wb_(w_tr), ALU.mult)
                    nc.vector.tensor_tensor(f3, f3, t3v, ALU.add)
                    nc.vector.tensor_tensor(t3v, corner(64), wb_(w_bl), ALU.mult)
                    nc.vector.tensor_tensor(f3, f3, t3v, ALU.add)
                    nc.vector.tensor_tensor(t3v, corner(192), wb_(w_br), ALU.mult)
                    nc.vector.tensor_tensor(f3, f3, t3v, ALU.add)

                    # transpose to channel-major: ot[2c+H, 128t+p] = ft[p, .]
                    ot = opool.tile([128, 2048], F32)
                    pt = tpsum.tile([128, 2048], BF16)
                    for t in range(16):
                        nc.tensor.transpose(
                            out=pt[:, 128 * t : 128 * (t + 1)],
                            in_=ft[:, 128 * t : 128 * (t + 1)],
                            identity=identb[:],
                        )
                    nc.scalar.copy(ot[:], pt[:])

                    # out[n, c, 2048H + f]; partition = 2c + H
                    dst = _ap(
                        out,
                        n * C * PIX,
                        [[PIX, 64], [2048, 2], [1, 2048]],
                    )
                    eng = nc.sync if n % 2 == 0 else nc.scalar
                    eng.dma_start(dst, ot[:])

    nc.compile()
    return nc


_NC_CACHE = None


def _get_nc():
    global _NC_CACHE
    if _NC_CACHE is None:
        _NC_CACHE = build_program()
    return _NC_CACHE


def make_in_maps(aer_feat, pose_uvr, offset_scale):
    in_maps = []
    for k in range(NCORES):
        b = k // (NCORES // B)
        n0 = (k % (NCORES // B)) * NP
        in_maps.append(
            {
                "img": np.ascontiguousarray(aer_feat[b]),
                "pose": np.ascontiguousarray(pose_uvr[b, n0 : n0 + NP]),
                "osc": np.ascontiguousarray(
                    offset_scale[b].reshape(1, 1).astype(np.float32)
                ),
            }
        )
    return in_maps


def assemble(results):
    full = np.empty((B, N, C, HB, WB), dtype=np.float32)
    for k in range(NCORES):
        b = k // (NCORES // B)
        n0 = (k % (NCORES // B)) * NP
        full[b, n0 : n0 + NP] = results[k]["out"].reshape(NP, C, HB, WB)
    return full


def kernel(aer_feat, pose_uvr, offset_scale):
    from concourse.bass_utils import run_bass_kernel_spmd

    nc = _get_nc()
    in_maps = make_in_maps(aer_feat, pose_uvr, offset_scale)
    res = run_bass_kernel_spmd(nc, in_maps, list(range(NCORES)))
    return assemble(res.results)



# revision 2
# speedup vs baseline: 1.0209x; 1.0209x over previous
"""AerialPatchSampler Trainium2 kernel (v3: two-level gather).

Samples N rotated/scaled/translated 64x64 patches from a (C=64, 512, 512)
aerial feature image with bilinear interpolation (grid_sample semantics,
align_corners=False, zeros padding + validity mask).

Sharding: 8 cores; core k handles batch b = k//4 and hypotheses
n in [32*(k%4), 32*(k%4)+32).  Each core receives its batch's full image.

Per-core plan:
  Phase A (unchanged from v2): build qimg[(Hp*Wp), 2*C] in DRAM in bf16:
    qimg[y*Wp+x] holds the channel vectors of padded rows y-1 and y at
    padded column x (zero border = grid_sample zero padding).
  Setup: coords pipeline on [64, 4096] (partition k = (patch n=k//2,
    corner row h=k%2), free = pixel i): slot coords sx=x0+1, sy=y0+1,
    weights, per-patch bbox mins (free-dim reduce), gather indices
    v = (sx-sxmin)*128 + (sy-symin) routed through DRAM into the
    [128, 256]-per-patch int16 layout dma_gather wants (replicated mod 16).
  Per patch:
    L1: ONE indirect DMA loads the 112x117-slot bbox of qimg into SBUF
        (112 descriptors x 29952B contiguous).
    L2: TWO SBUF-source dma_gathers (tpr=128, 256B ranks, elem=256B):
        left corners at v, right corners at v+128.  Output [128, 4096]
        bf16 with partition = h*64+c, column = pixel i.
    Weights: per-patch one-hot selector matmul broadcasts the [64, 4096]
        weight rows into [128, 1024] PSUM quarters.
    Combine: px[c,i] = sum_h sum_x w*corner via 3 bf16 mult/adds and a
        cross-partition-half add; ONE contiguous 1MB store per patch.
"""

import sys

for _p in ("/opt/trn_rl_repo", "/root/.axon_site/_ro/trn_rl_repo"):
    if _p not in sys.path:
        sys.path.insert(0, _p)

import os
import numpy as np

import concourse.bass as bass
import concourse.tile as tile
from concourse import bacc, mybir
from concourse.bass import AP
from concourse.masks import make_identity

F32 = mybir.dt.float32
BF16 = mybir.dt.bfloat16
I16 = mybir.dt.int16
I32 = mybir.dt.int32
ALU = mybir.AluOpType
ACTF = mybir.ActivationFunctionType

B, C, H, W = 2, 64, 512, 512
N = 128
HB, WB = 64, 64
NCORES = 8
NP = N // (NCORES // B)  # 32 patches per core
Hp, Wp = H + 2, W + 2  # 514, zero-padded
PV = Hp * Wp
PIX = HB * WB  # 4096
QR = 2 * C  # 128 elements per q slot
MAGIC = 12582912.0  # 1.5 * 2^23: RNE-to-int trick
RT = 32  # image rows per Phase-A tile
NT = H // RT  # 16 tiles
BROWS = 112   # bbox rows loaded per patch
BCOLS = 112   # bbox cols (256B slots) per partition
PAIROFF = BCOLS * 128  # rank offset of the pair's second patch
VMAX = float((BCOLS - 2) * 128 + (BROWS - 1))  # max left-corner index


def _ap(base: AP, extra_off: int, dims) -> AP:
    return AP(base.tensor, base.offset + extra_off, [list(d) for d in dims])


def build_program(repeat=1):
    nc = bacc.Bacc(
        "TRN2",
        target_bir_lowering=False,
        debug=False,
        enable_asserts=False,
        # the 16384-idx dma_gather needs 1026 ring-desc slots; the default
        # 16384B carveout caps at 1024
        dynamic_dma_scratch_size=18432,
    )
    img = nc.dram_tensor("img", [C, H, W], F32, kind="ExternalInput").ap()
    pose = nc.dram_tensor("pose", [NP, 3], F32, kind="ExternalInput").ap()
    osc = nc.dram_tensor("osc", [1, 1], F32, kind="ExternalInput").ap()
    out = nc.dram_tensor("out", [NP, C, PIX], F32, kind="ExternalOutput").ap()
    qimg = nc.dram_tensor("qimg", [PV, QR], BF16, kind="Internal").ap()
    vdramC = nc.dram_tensor("vdramC", [16, NP * 512], I16, kind="Internal").ap()
    bdram = nc.dram_tensor("bdram", [64, 1], F32, kind="Internal").ap()

    with tile.TileContext(nc, trace_sim=False) as tc:
      for _rep in range(repeat):
        with tc.tile_pool(name="const", bufs=1) as cpool:
            ident = cpool.tile([128, 128], F32)
            make_identity(nc, ident[:])

            zt = cpool.tile([128, 257], BF16)
            nc.vector.memset(zt[:], 0.0)

            # ---- persistent Phase-B tensors ----
            w4l = cpool.tile([64, PIX], BF16)   # (n,h) rows: left weights
            w4r = cpool.tile([64, PIX], BF16)   # right weights
            sel = cpool.tile([64, NP * 128], BF16)  # one-hot selectors
            fold = cpool.tile([128, 64], BF16)  # partition-half fold matrix
            idxsC = cpool.tile([128, NP * 512], I16)
            it32 = cpool.tile([128, NP], I32)   # L1 per-partition slot idx

            # ================= Setup: coords on [64, 4096] ================
            with tc.tile_pool(name="setup", bufs=1) as sp:
                def T(tag):
                    return sp.tile([64, PIX], F32, tag=tag, name=tag)

                # params dup'd to partition pairs: k = 2n + h -> pose[n]
                u2 = sp.tile([64, 1], F32)
                v2 = sp.tile([64, 1], F32)
                th2 = sp.tile([64, 1], F32)
                nc.sync.dma_start(u2[:], _ap(pose, 0, [[3, NP], [0, 2], [1, 1]]))
                nc.sync.dma_start(v2[:], _ap(pose, 1, [[3, NP], [0, 2], [1, 1]]))
                nc.sync.dma_start(th2[:], _ap(pose, 2, [[3, NP], [0, 2], [1, 1]]))
                os2 = sp.tile([64, 1], F32)
                nc.sync.dma_start(os2[:], _ap(osc, 0, [[0, 64], [1, 1]]))

                zb = sp.tile([64, 1], F32)
                nc.vector.memset(zb[:], 0.0)
                pib = sp.tile([64, 1], F32)
                nc.vector.memset(pib[:], 1.5707963267948966)
                eb = sp.tile([64, 1], F32)
                nc.vector.memset(eb[:], -255.5)

                sin2 = sp.tile([64, 1], F32)
                cos2 = sp.tile([64, 1], F32)
                ab2 = sp.tile([64, 1], F32)
                nc.scalar.activation(sin2[:], th2[:], ACTF.Sin, scale=-1.0,
                                     bias=zb[:, 0:1])
                nc.scalar.activation(ab2[:], th2[:], ACTF.Abs, bias=zb[:, 0:1])
                nc.scalar.activation(cos2[:], ab2[:], ACTF.Sin, scale=-1.0,
                                     bias=pib[:, 0:1])

                # parity h = k & 1 as f32; s1 = 2h-1, s0 = 1-h
                ki = sp.tile([64, 1], I32)
                nc.gpsimd.iota(ki[:], pattern=[[0, 1]], base=0,
                               channel_multiplier=1)
                hi2 = sp.tile([64, 1], I32)
                nc.vector.tensor_scalar(hi2[:], ki[:], 1, None,
                                        ALU.bitwise_and)
                h2 = sp.tile([64, 1], F32)
                nc.vector.tensor_copy(h2[:], hi2[:])
                s1 = sp.tile([64, 1], F32)
                nc.vector.tensor_scalar(s1[:], h2[:], 2.0, -1.0, ALU.mult,
                                        ALU.add)
                s0 = sp.tile([64, 1], F32)
                nc.vector.tensor_scalar(s0[:], h2[:], -1.0, 1.0, ALU.mult,
                                        ALU.add)

                # T0 = gu0 = (63-ii)*os ; T1 = gv0 = (jj-32)*os
                t0 = T("t0")
                nc.gpsimd.iota(t0[:], pattern=[[1, 64], [0, 64]], base=0,
                               channel_multiplier=0,
                               allow_small_or_imprecise_dtypes=True)
                nc.vector.tensor_scalar(t0[:], t0[:], -1.0, 63.0, ALU.mult,
                                        ALU.add)
                nc.vector.tensor_scalar(t0[:], t0[:], os2[:, 0:1], None,
                                        ALU.mult)
                t1 = T("t1")
                nc.gpsimd.iota(t1[:], pattern=[[0, 64], [1, 64]], base=0,
                               channel_multiplier=0,
                               allow_small_or_imprecise_dtypes=True)
                nc.vector.tensor_scalar(t1[:], t1[:], 32.0, os2[:, 0:1],
                                        ALU.subtract, ALU.mult)

                # T2 = xu ; T3 = yv
                t2 = T("t2")
                nc.vector.tensor_scalar(t2[:], t0[:], cos2[:, 0:1],
                                        u2[:, 0:1], ALU.mult, ALU.add)
                t3 = T("t3")
                nc.vector.tensor_scalar(t3[:], t1[:], sin2[:, 0:1], None,
                                        ALU.mult)
                nc.vector.tensor_tensor(t2[:], t2[:], t3[:], ALU.subtract)
                nc.vector.tensor_scalar(t3[:], t0[:], sin2[:, 0:1],
                                        v2[:, 0:1], ALU.mult, ALU.add)
                t4 = T("t4")
                nc.vector.tensor_scalar(t4[:], t1[:], cos2[:, 0:1], None,
                                        ALU.mult)
                nc.vector.tensor_tensor(t3[:], t3[:], t4[:], ALU.add)

                # T0 = valid (T5 = vx temp)
                t5 = T("t5")
                nc.scalar.activation(t4[:], t2[:], ACTF.Abs, bias=eb[:, 0:1])
                nc.vector.tensor_scalar(t5[:], t4[:], 256.0, None, ALU.is_lt)
                nc.scalar.activation(t4[:], t3[:], ACTF.Abs, bias=eb[:, 0:1])
                nc.vector.tensor_scalar(t0[:], t4[:], 256.0, None, ALU.is_lt)
                nc.vector.tensor_tensor(t0[:], t0[:], t5[:], ALU.mult)

                # T4 = x0f, T5 = wx (T2 free after) ; T1 = y0f, T2 = wy
                nc.vector.tensor_scalar(t4[:], t2[:], -0.5, MAGIC,
                                        ALU.add, ALU.add)
                nc.vector.tensor_scalar(t4[:], t4[:], -MAGIC, None, ALU.add)
                nc.vector.tensor_tensor(t5[:], t2[:], t4[:], ALU.subtract)
                nc.vector.tensor_scalar(t1[:], t3[:], -0.5, MAGIC,
                                        ALU.add, ALU.add)
                nc.vector.tensor_scalar(t1[:], t1[:], -MAGIC, None, ALU.add)
                nc.vector.tensor_tensor(t2[:], t3[:], t1[:], ALU.subtract)

                # T2 = fy = (wy*s1 + s0)*valid
                nc.vector.tensor_scalar(t2[:], t2[:], s1[:, 0:1], s0[:, 0:1],
                                        ALU.mult, ALU.add)
                nc.vector.tensor_tensor(t2[:], t2[:], t0[:], ALU.mult)
                # w4l = (1-wx)*fy ; w4r = wx*fy (bf16)
                t3_ = t3
                nc.vector.tensor_scalar(t3_[:], t5[:], -1.0, 1.0, ALU.mult,
                                        ALU.add)
                t6 = T("t6")
                nc.vector.tensor_tensor(t6[:], t3_[:], t2[:], ALU.mult)
                nc.vector.tensor_copy(w4l[:], t6[:])
                nc.vector.tensor_tensor(t6[:], t5[:], t2[:], ALU.mult)
                nc.vector.tensor_copy(w4r[:], t6[:])

                # T4 = sx = x0f+1 ; T1 = sy = y0f+1
                nc.vector.tensor_scalar(t4[:], t4[:], 1.0, None, ALU.add)
                nc.vector.tensor_scalar(t1[:], t1[:], 1.0, None, ALU.add)

                # mins over valid (T3 = big, T6 = masked)
                nc.vector.tensor_scalar(t3[:], t0[:], -1e6, 1e6, ALU.mult,
                                        ALU.add)
                sxmin = sp.tile([64, 1], F32)
                nc.vector.tensor_tensor(t6[:], t4[:], t3[:], ALU.add)
                nc.vector.tensor_reduce(sxmin[:], t6[:],
                                        mybir.AxisListType.X, ALU.min)
                symin = sp.tile([64, 1], F32)
                nc.vector.tensor_tensor(t6[:], t1[:], t3[:], ALU.add)
                nc.vector.tensor_reduce(symin[:], t6[:],
                                        mybir.AxisListType.X, ALU.min)

                # T4 = v = ((sx-sxmin)*128 + (sy-symin))*valid clamped
                nc.vector.tensor_scalar(t4[:], t4[:], sxmin[:, 0:1], None,
                                        ALU.subtract)
                nc.vector.tensor_scalar(t1[:], t1[:], symin[:, 0:1], None,
                                        ALU.subtract)
                nc.vector.scalar_tensor_tensor(t4[:], t4[:], 128.0, t1[:],
                                               ALU.mult, ALU.add)
                nc.vector.tensor_tensor(t4[:], t4[:], t0[:], ALU.mult)
                nc.vector.tensor_scalar(t4[:], t4[:], VMAX, 0.0, ALU.min,
                                        ALU.max)
                # combined L/R: row k = 2n+h gets v + 128*h (left corners
                # on even rows, right corners on odd rows)
                h128 = sp.tile([64, 1], F32)
                nc.vector.tensor_scalar(h128[:], h2[:], 128.0, None, ALU.mult)
                nc.vector.tensor_scalar(t4[:], t4[:], h128[:, 0:1], None,
                                        ALU.add)

                # vT[k, l*256+s] = v[k, s*16+l] (i16): lane dim l outer so
                # the replication read's final dim is contiguous
                vC = sp.tile([64, PIX], I16)
                vap = vC[:]
                nc.vector.tensor_copy(
                    _ap(vap, 0, [vap.ap[0], [256, 16], [1, 256]]),
                    _ap(t4[:], 0, [t4[:].ap[0], [1, 16], [16, 256]]),
                )

                # roundtrip: v -> DRAM [l, n, h, s] -> idxs [128, NP*512]
                nc.sync.dma_start(
                    _ap(vdramC, 0, [[256, 64], [NP * 512, 16], [1, 256]]),
                    vC[:])
                for q in range(8):
                    dsl = idxsC[16 * q:16 * q + 16, :]
                    nc.scalar.dma_start(
                        dsl,
                        _ap(vdramC, 0, [[NP * 512, 16], [1, NP * 512]]),
                    )

                # L1 base: base = symin*514 + sxmin (even k rows), roundtrip
                base = sp.tile([64, 1], F32)
                nc.vector.scalar_tensor_tensor(base[:], symin[:], float(Wp),
                                               sxmin[:], ALU.mult, ALU.add)
                nc.sync.dma_start(bdram, base[:])
                bbc = sp.tile([128, NP], F32)
                nc.sync.dma_start(bbc[:], _ap(bdram, 0, [[0, 128], [2, NP]]))
                p514 = sp.tile([128, 1], I32)
                nc.gpsimd.iota(p514[:], pattern=[[0, 1]], base=0,
                               channel_multiplier=Wp)
                p514f = sp.tile([128, 1], F32)
                nc.vector.tensor_copy(p514f[:], p514[:])
                itf = sp.tile([128, NP], F32)
                nc.vector.tensor_scalar(itf[:], bbc[:], p514f[:, 0:1], None,
                                        ALU.add)
                # clamp so idx + BCOLS stays inside qimg
                nc.vector.tensor_scalar(itf[:], itf[:], float(PV - BCOLS - 1),
                                        0.0, ALU.min, ALU.max)
                nc.vector.tensor_copy(it32[:], itf[:])

            # ============ Phase A: build double-row channel-last image =====
            ztap = zt[:]

            def zfill(eng, dst_off, nblk, blk_stride, blk_len):
                done = 0
                while done < nblk:
                    cnt = min(128, nblk - done)
                    eng.dma_start(
                        _ap(
                            qimg,
                            dst_off + done * blk_stride,
                            [[blk_stride, cnt], [1, blk_len]],
                        ),
                        _ap(ztap, 0, [[257, cnt], [1, blk_len]]),
                    )
                    done += cnt

            # col pads for q rows 0..512: slot (y, 513) + slot (y+1, 0)
            zfill(nc.scalar, (Wp - 1) * QR, H + 1, Wp * QR, 2 * QR)
            zfill(nc.scalar, 0, 1, QR, QR)  # slot (0, 0)
            # q row 0 low halves (pad image row): cols 1..512
            zfill(nc.sync, QR, W, QR, C)
            # q row 512 high halves (pad image row): cols 1..512
            zfill(nc.sync, (H * Wp + 1) * QR + C, W, QR, C)
            # q row 513: gathered only as bbox overrun; keep zeros
            zfill(nc.sync, (H + 1) * Wp * QR, Wp, QR, QR)

            with (
                tc.tile_pool(name="lpool", bufs=3) as lpool,
                tc.tile_pool(name="papsum", bufs=2, space="PSUM") as papsum,
                tc.tile_pool(name="spool", bufs=3) as spool,
            ):
                KP = RT // 2  # 16 row-pairs per tile
                prev_sb = None
                for t_i in range(NT):
                    base_r = t_i * RT  # rows [base_r, base_r+RT)
                    lt = lpool.tile([128, KP * 512], F32, tag="lt")
                    for y2 in (0, 1):
                        eng = nc.sync if y2 == 0 else nc.gpsimd
                        eng.dma_start(
                            lt[64 * y2 : 64 * y2 + 64, :],
                            _ap(
                                img,
                                (base_r + y2) * W,
                                [[H * W, C], [2 * W, KP], [1, W]],
                            ),
                        )
                    # sb[xl, k*2048 + r*64 + c] = img[c, base_r+r, 128k+xl]
                    sb = spool.tile([128, 4 * RT * C], BF16, tag="sb")
                    for q4 in range(KP // 4):
                        bt = papsum.tile([128, 2048], F32, tag="bt")
                        for pp in range(4):
                            p = 4 * q4 + pp
                            for k in range(4):
                                nc.tensor.transpose(
                                    out=bt[:, pp * 512 + 128 * k : pp * 512 + 128 * (k + 1)],
                                    in_=lt[:, p * 512 + 128 * k : p * 512 + 128 * (k + 1)],
                                    identity=ident[:],
                                )
                        sap = sb[:]
                        nc.vector.tensor_copy(
                            _ap(
                                sap,
                                512 * q4,
                                [sap.ap[0], [2048, 4], [128, 4], [1, 128]],
                            ),
                            _ap(bt[:], 0, [bt[:].ap[0], [128, 4], [512, 4], [1, 128]]),
                        )
                    sap = sb[:]
                    # whole slots y = base_r+1 .. base_r+31
                    store_engs = (nc.sync, nc.scalar, nc.scalar, nc.gpsimd)
                    for k in range(4):
                        eng = store_engs[k]
                        eng.dma_start(
                            _ap(
                                qimg,
                                ((base_r + 1) * Wp + 1 + 128 * k) * QR,
                                [[QR, 128], [Wp * QR, RT - 1], [1, QR]],
                            ),
                            _ap(
                                sap,
                                k * 2048,
                                [sap.ap[0], [64, RT - 1], [1, QR]],
                            ),
                        )
                    # boundary slot y = base_r
                    if prev_sb is not None:
                        pap = prev_sb[:]
                        nc.sync.dma_start(
                            _ap(
                                qimg,
                                (base_r * Wp + 1) * QR,
                                [[QR, 128], [128 * QR, 4], [1, C]],
                            ),
                            _ap(pap, (RT - 1) * 64, [pap.ap[0], [2048, 4], [1, C]]),
                        )
                    nc.sync.dma_start(
                        _ap(
                            qimg,
                            (base_r * Wp + 1) * QR + C,
                            [[QR, 128], [128 * QR, 4], [1, C]],
                        ),
                        _ap(sap, 0, [sap.ap[0], [2048, 4], [1, C]]),
                    )
                    if t_i == NT - 1:
                        nc.sync.dma_start(
                            _ap(
                                qimg,
                                (H * Wp + 1) * QR,
                                [[QR, 128], [128 * QR, 4], [1, C]],
                            ),
                            _ap(sap, (RT - 1) * 64, [sap.ap[0], [2048, 4], [1, C]]),
                        )
                    prev_sb = sb

            # build sel[k, n*128+p] = (k == 2n+(p>=64)), fold[p,c] = (p%64==c)
            with tc.tile_pool(name="selp", bufs=1) as selp:
                ks = selp.tile([64, NP * 128], I32)
                nc.gpsimd.iota(ks[:], pattern=[[2, NP], [1, 2], [0, 64]],
                               base=0, channel_multiplier=0)
                kk = selp.tile([64, 1], I32)
                nc.gpsimd.iota(kk[:], pattern=[[0, 1]], base=0,
                               channel_multiplier=1)
                ksf = selp.tile([64, NP * 128], F32)
                nc.vector.tensor_copy(ksf[:], ks[:])
                kkf = selp.tile([64, 1], F32)
                nc.vector.tensor_copy(kkf[:], kk[:])
                self_eq = selp.tile([64, NP * 128], F32)
                nc.vector.tensor_scalar(self_eq[:], ksf[:], kkf[:, 0:1], None,
                                        ALU.is_equal)
                nc.vector.tensor_copy(sel[:], self_eq[:])

                pc = selp.tile([128, 1], I32)
                nc.gpsimd.iota(pc[:], pattern=[[0, 1]], base=0,
                               channel_multiplier=1)
                pm = selp.tile([128, 1], I32)
                nc.vector.tensor_scalar(pm[:], pc[:], 63, None,
                                        ALU.bitwise_and)
                pmf = selp.tile([128, 1], F32)
                nc.vector.tensor_copy(pmf[:], pm[:])
                cc = selp.tile([128, 64], I32)
                nc.gpsimd.iota(cc[:], pattern=[[1, 64]], base=0,
                               channel_multiplier=0)
                ccf = selp.tile([128, 64], F32)
                nc.vector.tensor_copy(ccf[:], cc[:])
                feq = selp.tile([128, 64], F32)
                nc.vector.tensor_scalar(feq[:], ccf[:], pmf[:, 0:1], None,
                                        ALU.is_equal)
                nc.vector.tensor_copy(fold[:], feq[:])

            # ================= Phase B: per-patch sample ===================
            with (
                tc.tile_pool(name="boxp", bufs=2) as boxp,
                tc.tile_pool(name="gpool", bufs=2) as gpool,
                tc.tile_pool(name="wpsum", bufs=1, space="PSUM") as wpsum,
                tc.tile_pool(name="xpsum", bufs=2, space="PSUM") as xpsum,
                tc.tile_pool(name="opool", bufs=2) as opool,
            ):
                for n in range(NP):
                    bbox = boxp.tile([BROWS, BCOLS * 128], BF16, tag="bbox")
                    if not os.environ.get("K2_SKIP_L1"):
                        nc.gpsimd.indirect_dma_start(
                            out=bbox[:],
                            out_offset=None,
                            in_=qimg,
                            in_offset=bass.IndirectOffsetOnAxis(
                                ap=it32[0:BROWS, n:n + 1], axis=0
                            ),
                        )

                    gC = gpool.tile([128, 2 * PIX], BF16, tag="gC")
                    gap = gC[:]
                    if not os.environ.get("K2_SKIP_GATHER"):
                        nc.gpsimd.dma_gather(
                            out_ap=AP(gap.tensor, gap.offset,
                                      [list(gap.ap[0]), [2 * PIX, 1],
                                       [1, 2 * PIX]]),
                            in_ap=bbox[:],
                            idxs_ap=idxsC[:, n * 512:(n + 1) * 512],
                            num_idxs=2 * PIX,
                            num_idxs_reg=2 * PIX,
                            elem_size=128,
                            transpose=True,
                            single_packet=False,
                            sbuf_tokens_per_rank=128,
                            sbuf_free_dim_per_rank=256,
                            sbuf_free_dim_pad_per_rank=0,
                            sbuf_byte_offset=0,
                        )
                    else:
                        nc.vector.memset(gC[:], 0.0)

                    px = opool.tile([64, PIX], F32, tag="px")
                    for qq in range(4):
                        s = slice(qq * 1024, (qq + 1) * 1024)
                        pL = wpsum.tile([128, 1024], F32, tag="pL")
                        pR = wpsum.tile([128, 1024], F32, tag="pR")
                        for hh in (0, 1):
                            sh = slice(qq * 1024 + hh * 512,
                                       qq * 1024 + (hh + 1) * 512)
                            nc.tensor.matmul(pL[:, hh * 512:(hh + 1) * 512],
                                             sel[:, n * 128:(n + 1) * 128],
                                             w4l[:, sh], start=True, stop=True)
                            nc.tensor.matmul(pR[:, hh * 512:(hh + 1) * 512],
                                             sel[:, n * 128:(n + 1) * 128],
                                             w4r[:, sh], start=True, stop=True)
                        tmY = gpool.tile([128, 1024], BF16, tag="tmY")
                        tmZ = gpool.tile([128, 1024], BF16, tag="tmZ")
                        sR = slice(PIX + qq * 1024, PIX + (qq + 1) * 1024)
                        nc.vector.tensor_tensor(tmY[:], gC[:, s], pL[:],
                                                ALU.mult)
                        nc.vector.tensor_tensor(tmZ[:], gC[:, sR], pR[:],
                                                ALU.mult)
                        nc.vector.tensor_tensor(tmY[:], tmY[:], tmZ[:],
                                                ALU.add)
                        # fold partition halves: px[c] = tm[c] + tm[64+c]
                        pxp = xpsum.tile([64, 1024], F32, tag="pxp")
                        for hh in (0, 1):
                            nc.tensor.matmul(pxp[:, hh * 512:(hh + 1) * 512],
                                             fold[:],
                                             tmY[:, hh * 512:(hh + 1) * 512],
                                             start=True, stop=True)
                        nc.vector.tensor_copy(px[:, s], pxp[:])

                    dst = _ap(out, n * C * PIX, [[PIX, C], [1, PIX]])
                    eng = nc.sync if n % 2 == 0 else nc.scalar
                    eng.dma_start(dst, px[:])

    nc.compile()
    return nc


_NC_CACHE = None


def _get_nc():
    global _NC_CACHE
    if _NC_CACHE is None:
        _NC_CACHE = build_program(int(os.environ.get("K2_REPEAT", "1")))
    return _NC_CACHE


def make_in_maps(aer_feat, pose_uvr, offset_scale):
    in_maps = []
    for k in range(NCORES):
        b = k // (NCORES // B)
        n0 = (k % (NCORES // B)) * NP
        in_maps.append(
            {
                "img": np.ascontiguousarray(aer_feat[b]),
                "pose": np.ascontiguousarray(pose_uvr[b, n0 : n0 + NP]),
                "osc": np.ascontiguousarray(
                    offset_scale[b].reshape(1, 1).astype(np.float32)
                ),
            }
        )
    return in_maps


def assemble(results):
    full = np.empty((B, N, C, HB, WB), dtype=np.float32)
    for k in range(NCORES):
        b = k // (NCORES // B)
        n0 = (k % (NCORES // B)) * NP
        full[b, n0 : n0 + NP] = results[k]["out"].reshape(NP, C, HB, WB)
    return full


def kernel(aer_feat, pose_uvr, offset_scale):
    from concourse.bass_utils import run_bass_kernel_spmd

    nc = _get_nc()
    in_maps = make_in_maps(aer_feat, pose_uvr, offset_scale)
    res = run_bass_kernel_spmd(nc, in_maps, list(range(NCORES)))
    return assemble(res.results)
